# revision 1
# baseline (speedup 1.0000x reference)
"""GCN (DGL GraphConv x3 + residual + FC) on 8 Trainium2 NeuronCores.

Sharding: nodes are range-partitioned across the 8 cores (6250 nodes each).
Each core owns the edges whose dst falls in its shard.  Per layer, every core
computes the dense transform for its node shard (feat-major activations so no
transposes are ever needed), all-gathers the resulting 50000x128 bf16 message
table, gathers its edges' source rows with dma_gather (edge-major [128e,128f]
tiles), and segment-sums them into PSUM via one-hot matmuls (edges are
pre-sorted by dst on the host, so each 128-dst block accumulates a handful of
edge tiles).  Degree scalings fold in at node granularity:
  out[d] = r_in[d] * sum_e  (x W)[src_e] * r_out[src_e]   (+ bias, relu)
Key simplification: the reference computes relu(gconv(x1,W2)) twice (branch
and main are identical), so only 3 graph convs are needed, and the final
relu(x3+x2) is the identity on already-relu'd tensors (x3,x2 >= 0).

dma_gather indices are int16, so the table is split into two 25000-row halves;
each core keeps two dst-sorted edge lists (src<25000 / src>=25000) padded
per (block, half) to a common tile count across cores so all 8 cores run the
same program (SPMD) with different data.
"""
import sys

sys.path.insert(0, "/opt/trn_rl_repo")

import numpy as np
import ml_dtypes

from concourse import bacc, mybir, tile
from concourse.bass_utils import run_bass_kernel_spmd

BF16 = ml_dtypes.bfloat16
F32 = mybir.dt.float32
BF = mybir.dt.bfloat16
I16 = mybir.dt.int16

N_NODES = 50000
N_EDGES = 600000
IN_F = 602
HID = 128
OUT_F = 41
NCORES = 8
SH = N_NODES // NCORES          # 6250 nodes per core
P = 128
NBLK = (SH + P - 1) // P        # 49 dst blocks (last has 106)
LASTM = SH - (NBLK - 1) * P     # 106
HALF = N_NODES // 2             # table half split for int16 indices
KCH = 5                         # ceil(602/128) k-chunks for layer 1
INF_PAD = KCH * P               # 640
CHUNK_TILES = 8                 # edge tiles per dma_gather call (1024 edges; >=2048 faults on HW)
ST_GROUP = 8                    # edge tiles per one-hot DVE op
PAD_SLOT = 1000.0               # one-hot compare value for pad slots (never matches)


# ----------------------------------------------------------------- host prep

def _wrap_idx16(idx):
    """dma_gather idx layout: elem i -> partition i%16, slot i//16; replicated
    to 128 partitions (8 gpsimd cores read identical copies)."""
    n = len(idx)
    w = np.asarray(idx, np.int16).reshape(n // 16, 16).T
    return np.tile(w, (8, 1))


def _preprocess(features, src, dst, W1, b1, W2, b2, W3, b3, Wfc, bfc):
    src = np.asarray(src).astype(np.int64)
    dst = np.asarray(dst).astype(np.int64)
    features = np.asarray(features, np.float32)

    core_of = dst // SH
    per_core = []  # (idxA, slotA, idxB, slotB) unpadded, per (block, half)
    nA = np.zeros((NCORES, NBLK), np.int64)
    nB = np.zeros((NCORES, NBLK), np.int64)
    for c in range(NCORES):
        sel = core_of == c
        s = src[sel]
        dl = dst[sel] - c * SH
        order = np.argsort(dl, kind="stable")
        s, dl = s[order], dl[order]
        blk = dl >> 7
        slot = dl & 127
        isA = s < HALF
        blocksA, blocksB = [], []
        for b in range(NBLK):
            inb = blk == b
            a = inb & isA
            bb = inb & ~isA
            blocksA.append((s[a], slot[a]))
            blocksB.append((s[bb] - HALF, slot[bb]))
            nA[c, b] = a.sum()
            nB[c, b] = bb.sum()
        per_core.append((blocksA, blocksB))

    # common tile counts per (block, half) across cores
    TA = np.maximum(1, np.ceil(nA.max(0) / P).astype(np.int64))
    TB = np.maximum(1, np.ceil(nB.max(0) / P).astype(np.int64))
    TA_tot, TB_tot = int(TA.sum()), int(TB.sum())

    def build_half(blocks, T):
        idx = np.zeros(int(T.sum()) * P, np.int16)
        slot = np.full(int(T.sum()) * P, PAD_SLOT, np.float32)
        off = 0
        for b in range(NBLK):
            i, sl = blocks[b]
            n = len(i)
            idx[off:off + n] = i
            slot[off:off + n] = sl
            off += int(T[b]) * P
        return idx, slot

    in_maps = []
    deg_out_full = np.bincount(src, minlength=N_NODES).astype(np.float32)
    cum_out = np.concatenate([[0.0], np.cumsum(deg_out_full)]).astype(np.float32)

    featT = np.zeros((INF_PAD, N_NODES), np.float32)
    featT[:IN_F] = features.T
    W1p = np.zeros((INF_PAD, HID), np.float32)
    W1p[:IN_F] = W1

    for c in range(NCORES):
        blocksA, blocksB = per_core[c]
        idxA, slotA = build_half(blocksA, TA)
        idxB, slotB = build_half(blocksB, TB)
        slotAB = np.concatenate([slotA, slotB]).reshape(TA_tot + TB_tot, P).T

        # in-degree bounds (dst-sorted cumulative positions), this core's shard
        deg_in = np.bincount(dst[core_of == c] - c * SH, minlength=SH)
        cum_in = np.concatenate([[0], np.cumsum(deg_in)]).astype(np.float32)

        # out-degree bounds for this shard, node-partition-wrapped [128, NBLK]
        lo = np.zeros(NBLK * P, np.float32)
        hi = np.zeros(NBLK * P, np.float32)
        lo[:SH] = cum_out[c * SH: (c + 1) * SH]
        hi[:SH] = cum_out[c * SH + 1: (c + 1) * SH + 1]
        b_out_lo = lo.reshape(NBLK, P).T.copy()
        b_out_hi = hi.reshape(NBLK, P).T.copy()

        # layer-1 features, block-contiguous: [NBLK, 128, KCH, 128]
        fb = np.zeros((NBLK, P, KCH, P), np.float32)
        shard = featT[:, c * SH:(c + 1) * SH]
        for nb in range(NBLK):
            m = P if nb < NBLK - 1 else LASTM
            fb[nb, :, :, :m] = (shard[:, nb * P: nb * P + m]
                                .reshape(KCH, P, m).transpose(1, 0, 2))

        in_maps.append({
            "featB": fb.astype(BF16),
            "W1c": W1p.reshape(KCH, P, HID).transpose(1, 0, 2).astype(BF16).copy(),
            "W2c": W2.astype(BF16), "W3c": W3.astype(BF16),
            "Wfcc": Wfc.astype(BF16),
            "b1c": b1.reshape(HID, 1).astype(np.float32),
            "b2c": b2.reshape(HID, 1).astype(np.float32),
            "b3c": b3.reshape(HID, 1).astype(np.float32),
            "bfcc": bfc.reshape(1, OUT_F).astype(BF16),
            "iota": np.tile(np.arange(P, dtype=np.float32).astype(BF16), (P, 1)),
            "ones_f": np.ones((1, P), np.float32),
            "ones_b": np.ones((1, P), BF16),
            "idxA": _wrap_idx16(idxA), "idxB": _wrap_idx16(idxB),
            "slotAB": slotAB.astype(BF16),
            "b_in_lo": cum_in[:SH].reshape(1, SH),
            "b_in_hi": cum_in[1:SH + 1].reshape(1, SH),
            "b_out_lo": b_out_lo, "b_out_hi": b_out_hi,
        })

    sched = {"TA": TA.tolist(), "TB": TB.tolist(),
             "TA_tot": TA_tot, "TB_tot": TB_tot}
    return in_maps, sched


# ------------------------------------------------------------- device program

def _build(sched):
    TA, TB = sched["TA"], sched["TB"]
    TA_tot, TB_tot = sched["TA_tot"], sched["TB_tot"]
    EA, EB = TA_tot * P, TB_tot * P

    nc = bacc.Bacc("TRN2", target_bir_lowering=False, debug=False,
                   num_devices=NCORES, num_swdge_queues=4)

    def din(name, shape, dt):
        return nc.dram_tensor(name, shape, dt, kind="ExternalInput")

    h = {
        "featB": din("featB", [NBLK, P, KCH, P], BF),
        "W1c": din("W1c", [P, KCH, HID], BF),
        "W2c": din("W2c", [HID, HID], BF),
        "W3c": din("W3c", [HID, HID], BF),
        "Wfcc": din("Wfcc", [HID, OUT_F], BF),
        "b1c": din("b1c", [HID, 1], F32),
        "b2c": din("b2c", [HID, 1], F32),
        "b3c": din("b3c", [HID, 1], F32),
        "bfcc": din("bfcc", [1, OUT_F], BF),
        "iota": din("iota", [P, P], BF),
        "ones_f": din("ones_f", [1, P], F32),
        "ones_b": din("ones_b", [1, P], BF),
        "idxA": din("idxA", [P, EA // 16], I16),
        "idxB": din("idxB", [P, EB // 16], I16),
        "slotAB": din("slotAB", [P, TA_tot + TB_tot], BF),
        "b_in_lo": din("b_in_lo", [1, SH], F32),
        "b_in_hi": din("b_in_hi", [1, SH], F32),
        "b_out_lo": din("b_out_lo", [P, NBLK], F32),
        "b_out_hi": din("b_out_hi", [P, NBLK], F32),
    }
    out_fc = nc.dram_tensor("out_fc", [SH, OUT_F], F32, kind="ExternalOutput")
    Tshard = nc.dram_tensor("Tshard", [SH, HID], BF)
    Tfull = nc.dram_tensor("Tfull", [N_NODES, HID], BF, addr_space="Shared")

    with tile.TileContext(nc) as tc, \
         tc.tile_pool(name="const", bufs=1) as cp, \
         tc.tile_pool(name="state", bufs=1) as statep, \
         tc.tile_pool(name="feat", bufs=2 * KCH) as featp, \
         tc.tile_pool(name="msgA", bufs=10) as msgAp, \
         tc.tile_pool(name="msgB", bufs=10) as msgBp, \
         tc.tile_pool(name="sTA", bufs=6) as sTAp, \
         tc.tile_pool(name="sTB", bufs=6) as sTBp, \
         tc.tile_pool(name="stage", bufs=3) as stagep, \
         tc.tile_pool(name="ps_agg", bufs=4, space="PSUM") as ps_agg, \
         tc.tile_pool(name="ps_dense", bufs=2, space="PSUM") as ps_dense, \
         tc.tile_pool(name="ps_misc", bufs=1, space="PSUM") as ps_misc, \
         tc.tile_pool(name="ps_fc", bufs=1, space="PSUM") as ps_fc:

        def load(name, shape, dt):
            t = cp.tile(shape, dt, tag=name)
            nc.sync.dma_start(out=t[:], in_=h[name][:])
            return t

        W1s = load("W1c", [P, KCH, HID], BF)
        W2s = load("W2c", [HID, HID], BF)
        W3s = load("W3c", [HID, HID], BF)
        Wfcs = load("Wfcc", [HID, OUT_F], BF)
        b1s = load("b1c", [HID, 1], F32)
        b2s = load("b2c", [HID, 1], F32)
        b3s = load("b3c", [HID, 1], F32)
        bfcs = load("bfcc", [1, OUT_F], BF)
        iota = load("iota", [P, P], BF)
        ones_f = load("ones_f", [1, P], F32)
        ones_b = load("ones_b", [1, P], BF)
        idxA = load("idxA", [P, EA // 16], I16)
        idxB = load("idxB", [P, EB // 16], I16)
        slotAB = load("slotAB", [P, TA_tot + TB_tot], BF)
        bol = load("b_out_lo", [P, NBLK], F32)
        boh = load("b_out_hi", [P, NBLK], F32)

        # degrees -> r = 1/sqrt(max(deg,1))
        r_out = cp.tile([P, NBLK], F32, tag="r_out")
        nc.vector.tensor_sub(out=r_out[:], in0=boh[:], in1=bol[:])
        nc.vector.tensor_scalar_max(out=r_out[:], in0=r_out[:], scalar1=1.0)
        nc.scalar.activation(r_out[:], r_out[:], mybir.ActivationFunctionType.Sqrt)
        nc.vector.reciprocal(out=r_out[:], in_=r_out[:])

        # r_in = 1/sqrt(max(deg_in,1)), computed in 512-col chunks and
        # broadcast to [128, SH] via K=1 matmuls
        r_in_b = statep.tile([P, SH], F32, tag="r_in_b")
        for j in range(0, SH, 512):
            w = min(512, SH - j)
            blo = featp.tile([1, 512], F32, tag="blo")
            bhi = featp.tile([1, 512], F32, tag="bhi")
            nc.sync.dma_start(out=blo[:, :w], in_=h["b_in_lo"][:, j:j + w])
            nc.sync.dma_start(out=bhi[:, :w], in_=h["b_in_hi"][:, j:j + w])
            nc.vector.tensor_sub(out=blo[:, :w], in0=bhi[:, :w], in1=blo[:, :w])
            nc.vector.tensor_scalar_max(out=blo[:, :w], in0=blo[:, :w], scalar1=1.0)
            nc.scalar.activation(blo[:, :w], blo[:, :w],
                                 mybir.ActivationFunctionType.Sqrt)
            nc.vector.reciprocal_approx_fast(out=blo[:, :w], in_=blo[:, :w])
            pm = ps_misc.tile([P, 512], F32, space="PSUM")
            nc.tensor.matmul(pm[:, :w], lhsT=ones_f[:], rhs=blo[:, :w],
                             start=True, stop=True)
            nc.vector.tensor_copy(out=r_in_b[:, j:j + w], in_=pm[:, :w])

        agg = statep.tile([P, SH], F32, tag="agg")
        x1 = statep.tile([P, SH], BF, tag="x1")
        x2 = statep.tile([P, SH], BF, tag="x2")

        # ---- T1 = r_out * (features @ W1), feature tiles streamed from DRAM
        for nb in range(NBLK):
            m = P if nb < NBLK - 1 else LASTM
            ft = featp.tile([P, KCH, P], BF, tag="ft")
            nc.sync.dma_start(out=ft[:], in_=h["featB"][nb])
            ps = ps_dense.tile([P, HID], F32, space="PSUM")
            for k in range(KCH):
                nc.tensor.matmul(ps[:m, :],
                                 lhsT=ft[:, k, :m],
                                 rhs=W1s[:, k, :],
                                 start=(k == 0), stop=(k == KCH - 1))
            st = stagep.tile([P, HID], BF, tag="st")
            nc.vector.tensor_scalar_mul(out=st[:m, :], in0=ps[:m, :],
                                        scalar1=r_out[:m, nb:nb + 1])
            nc.scalar.dma_start(out=Tshard[nb * P:nb * P + m, :], in_=st[:m, :])
        nc.gpsimd.collective_compute(
            "AllGather", mybir.AluOpType.bypass,
            replica_groups=[list(range(NCORES))],
            ins=[Tshard[:].opt()], outs=[Tfull[:].opt()])

        # ---- aggregation machinery
        def emit_gather(tot_tiles, idx_t, base_ap, pool, k, q):
            nt = min(CHUNK_TILES, tot_tiles - k * CHUNK_TILES)
            mt = pool.tile([P, nt, HID], BF, tag="msg")
            nidx = nt * P
            c0 = k * CHUNK_TILES * P // 16
            nc.gpsimd.dma_gather(
                out_ap=mt[:], in_ap=base_ap, idxs_ap=idx_t[:, c0:c0 + nidx // 16],
                num_idxs=nidx, num_idxs_reg=nidx, elem_size=HID,
                queue_num=q % 4)
            return mt

        def gathers_interleaved():
            """Emit A/B gather calls alternating (matches per-block A-then-B
            consumption order) round-robin across the 8 SWDGE queues."""
            ncA = (TA_tot + CHUNK_TILES - 1) // CHUNK_TILES
            ncB = (TB_tot + CHUNK_TILES - 1) // CHUNK_TILES
            mA, mB = [], []
            q = 0
            for k in range(max(ncA, ncB)):
                if k < ncA:
                    mA.append(emit_gather(TA_tot, idxA, Tfull[0:HALF, :], msgAp, k, q))
                    q += 1
                if k < ncB:
                    mB.append(emit_gather(TB_tot, idxB, Tfull[HALF:N_NODES, :], msgBp, k, q))
                    q += 1
            return mA, mB

        def st_group(pool, slot_off, g, tot_tiles):
            nt = min(ST_GROUP, tot_tiles - g * ST_GROUP)
            t = pool.tile([P, ST_GROUP, P], BF, tag="sT")
            sl = slotAB[:, slot_off + g * ST_GROUP: slot_off + g * ST_GROUP + nt]
            nc.vector.tensor_tensor(
                out=t[:, :nt, :],
                in0=sl.unsqueeze(2).to_broadcast([P, nt, P]),
                in1=iota[:].unsqueeze(1).to_broadcast([P, nt, P]),
                op=mybir.AluOpType.is_equal)
            return t

        def aggregate(bias, xout):
            mA, mB = gathers_interleaved()
            sA, sB = {}, {}
            tA = tB = 0
            for b in range(NBLK):
                ps = ps_agg.tile([P, P], F32, space="PSUM")
                tot = TA[b] + TB[b]
                i = 0
                for (cnt, cur, msgs, sTs, pool, soff, ttot) in (
                        (TA[b], tA, mA, sA, sTAp, 0, TA_tot),
                        (TB[b], tB, mB, sB, sTBp, TA_tot, TB_tot)):
                    for t in range(cur, cur + cnt):
                        g = t // ST_GROUP
                        if g not in sTs:
                            sTs[g] = st_group(pool, soff, g, ttot)
                        nc.tensor.matmul(
                            ps[:],
                            lhsT=msgs[t // CHUNK_TILES][:, t % CHUNK_TILES, :],
                            rhs=sTs[g][:, t % ST_GROUP, :],
                            start=(i == 0), stop=(i == tot - 1))
                        i += 1
                tA += TA[b]
                tB += TB[b]
                m = P if b < NBLK - 1 else LASTM
                nc.vector.tensor_mul(out=agg[:, b * P:b * P + m], in0=ps[:, :m],
                                     in1=r_in_b[:, b * P:b * P + m])
            nc.scalar.activation(xout[:], agg[:],
                                 mybir.ActivationFunctionType.Relu, bias=bias[:])

        def dense_to_table(xin, Wt):
            for nb in range(NBLK):
                m = P if nb < NBLK - 1 else LASTM
                ps = ps_dense.tile([P, HID], F32, space="PSUM")
                nc.tensor.matmul(ps[:m, :], lhsT=xin[:, nb * P:nb * P + m],
                                 rhs=Wt[:], start=True, stop=True)
                st = stagep.tile([P, HID], BF, tag="st")
                nc.vector.tensor_scalar_mul(out=st[:m, :], in0=ps[:m, :],
                                            scalar1=r_out[:m, nb:nb + 1])
                nc.scalar.dma_start(out=Tshard[nb * P:nb * P + m, :], in_=st[:m, :])
            nc.gpsimd.collective_compute(
                "AllGather", mybir.AluOpType.bypass,
                replica_groups=[list(range(NCORES))],
                ins=[Tshard[:].opt()], outs=[Tfull[:].opt()])

        aggregate(b1s, x1)          # x1 = relu(gconv(features, W1))
        dense_to_table(x1, W2s)     # T2
        aggregate(b2s, x2)          # x2 = relu(gconv(x1, W2))
        dense_to_table(x2, W3s)     # T3
        x3 = statep.tile([P, SH], BF, tag="x1")   # reuse x1 slot
        aggregate(b3s, x3)          # x3 = relu(gconv(x2, W3))

        # x4 = relu(x3 + x2) == x3 + x2 (both already >= 0); in-place into x2
        x4 = x2
        nc.vector.tensor_add(out=x4[:], in0=x3[:], in1=x2[:])

        # out = x4 @ Wfc + bfc
        for nb in range(NBLK):
            m = P if nb < NBLK - 1 else LASTM
            ps = ps_fc.tile([P, OUT_F], F32, space="PSUM")
            nc.tensor.matmul(ps[:m, :], lhsT=x4[:, nb * P:nb * P + m],
                             rhs=Wfcs[:], start=True, stop=False)
            nc.tensor.matmul(ps[:m, :], lhsT=ones_b[:, :m], rhs=bfcs[:],
                             start=False, stop=True)
            st = stagep.tile([P, OUT_F], F32, tag="stf")
            nc.vector.tensor_copy(out=st[:m, :], in_=ps[:m, :])
            nc.sync.dma_start(out=out_fc[nb * P:nb * P + m, :], in_=st[:m, :])

    nc.compile()
    return nc


_CACHED = None


def kernel(**inputs):
    global _CACHED
    in_maps, sched = _preprocess(**inputs)
    if _CACHED is None or _CACHED[1] != sched:
        _CACHED = (_build(sched), sched)
    nc = _CACHED[0]
    res = run_bass_kernel_spmd(nc, in_maps, list(range(NCORES)))
    return np.concatenate(
        [np.asarray(res.results[c]["out_fc"], np.float32) for c in range(NCORES)], 0)



# revision 7
# speedup vs baseline: 1.0258x; 1.0258x over previous
"""GCN (DGL GraphConv x3 + residual + FC) on 8 Trainium2 NeuronCores.

Sharding: nodes range-partitioned across 8 cores (6250 each); each core owns
edges whose dst falls in its shard.  Per layer every core computes the dense
transform for its shard (node-major table rows), all-gathers the 50000x128
bf16 message table, gathers its edges' source rows with dma_gather
(edge-major [128e,128f] tiles), and segment-sums them into PSUM via one-hot
matmuls (edges pre-sorted by dst on the host).

This version software-pipelines the whole network at dst-block granularity:
  - The table is stored in a remapped node order so that the first half
    (shard rows [0,3125) of every core) is contiguous; each AllGather is
    split into two half-table collectives fired from the scalar queue as
    soon as the corresponding half of the shard's dense rows are written.
    Collectives hide behind the gather stream instead of idling the chip.
  - The next layer's dense transform is fused into the aggregation loop
    (lagged 2 blocks), so AG(l+1) + the layer-(l+1) gathers launch while
    layer l's gather stream is still draining: the 4 SWDGE gather queues
    (the ~100GB/s bottleneck) stay continuously fed across layers.
  - Tshard/Tfull are double-buffered across layers to break WAR hazards.
  - FC head is computed feature-major (stationary Wfc, 512-col moving
    streams, bias via activation); the host transposes the [41,6250] shard
    outputs back.
Key simplification: the reference computes relu(gconv(x1,W2)) twice (branch
and main are identical) so only 3 graph convs are needed, and relu(x3+x2) is
the identity on already-relu'd tensors.

dma_gather indices are int16 so the table is split into two 25000-row
halves (A = shard rows [0,3125) of all cores, B = the rest); each core keeps
two dst-sorted edge lists padded per (block, half) to a common tile count
across cores so all 8 cores run the same SPMD program.
"""
import sys

sys.path.insert(0, "/opt/trn_rl_repo")

import numpy as np
import ml_dtypes

from concourse import bacc, mybir, tile
from concourse.bass_utils import run_bass_kernel_spmd

BF16 = ml_dtypes.bfloat16
F32 = mybir.dt.float32
BF = mybir.dt.bfloat16
I16 = mybir.dt.int16

N_NODES = 50000
N_EDGES = 600000
IN_F = 602
HID = 128
OUT_F = 41
NCORES = 8
SH = N_NODES // NCORES          # 6250 nodes per core
HSH = SH // 2                   # 3125: half-shard boundary for split AG
P = 128
NBLK = (SH + P - 1) // P        # 49 dst blocks (last has 106)
LASTM = SH - (NBLK - 1) * P     # 106
HALF = N_NODES // 2             # 25000-row table halves (int16 idx range)
KCH = 5                         # ceil(602/128) k-chunks for layer 1
INF_PAD = KCH * P               # 640
CHUNK_TILES = 8                 # edge tiles per dma_gather call (1024 edges)
ST_GROUP = 8                    # edge tiles per one-hot DVE op
PAD_SLOT = 1000.0               # one-hot compare value for pad slots
LAG = 2                         # blocks of lag between agg and next dense
AG_BLK = 26                     # agg-loop index at which half-1 AG fires


# ----------------------------------------------------------------- host prep

def _wrap_idx16(idx):
    """dma_gather idx layout: elem i -> partition i%16, slot i//16; replicated
    to 128 partitions (8 gpsimd cores read identical copies)."""
    n = len(idx)
    w = np.asarray(idx, np.int16).reshape(n // 16, 16).T
    return np.tile(w, (8, 1))


def _preprocess(features, src, dst, W1, b1, W2, b2, W3, b3, Wfc, bfc):
    src = np.asarray(src).astype(np.int64)
    dst = np.asarray(dst).astype(np.int64)
    features = np.asarray(features, np.float32)

    # Table node order remap: node v (core c=v//SH, row r=v%SH) sits at
    #   A half (r < HSH):  pos = c*HSH + r
    #   B half (r >= HSH): pos = c*HSH + (r - HSH)   [within the B region]
    # which is exactly the concat layout of AllGather(Tshard[0:HSH]) and
    # AllGather(Tshard[HSH:SH]).
    core_of = dst // SH
    per_core = []  # (blocksA, blocksB) unpadded, per (block, half)
    nA = np.zeros((NCORES, NBLK), np.int64)
    nB = np.zeros((NCORES, NBLK), np.int64)
    for c in range(NCORES):
        sel = core_of == c
        s = src[sel]
        dl = dst[sel] - c * SH
        order = np.argsort(dl, kind="stable")
        s, dl = s[order], dl[order]
        blk = dl >> 7
        slot = dl & 127
        sc = s // SH
        sr = s % SH
        isA = sr < HSH
        posA = sc * HSH + sr          # valid where isA
        posB = sc * HSH + (sr - HSH)  # valid where ~isA
        blocksA, blocksB = [], []
        for b in range(NBLK):
            inb = blk == b
            a = inb & isA
            bb = inb & ~isA
            blocksA.append((posA[a], slot[a]))
            blocksB.append((posB[bb], slot[bb]))
            nA[c, b] = a.sum()
            nB[c, b] = bb.sum()
        per_core.append((blocksA, blocksB))

    # common tile counts per (block, half) across cores
    TA = np.maximum(1, np.ceil(nA.max(0) / P).astype(np.int64))
    TB = np.maximum(1, np.ceil(nB.max(0) / P).astype(np.int64))
    TA_tot, TB_tot = int(TA.sum()), int(TB.sum())

    def build_half(blocks, T):
        idx = np.zeros(int(T.sum()) * P, np.int16)
        slot = np.full(int(T.sum()) * P, PAD_SLOT, np.float32)
        off = 0
        for b in range(NBLK):
            i, sl = blocks[b]
            n = len(i)
            idx[off:off + n] = i
            slot[off:off + n] = sl
            off += int(T[b]) * P
        return idx, slot

    in_maps = []
    deg_out_full = np.bincount(src, minlength=N_NODES).astype(np.float32)
    cum_out = np.concatenate([[0.0], np.cumsum(deg_out_full)]).astype(np.float32)

    featT = np.zeros((INF_PAD, N_NODES), np.float32)
    featT[:IN_F] = features.T
    W1p = np.zeros((INF_PAD, HID), np.float32)
    W1p[:IN_F] = W1

    for c in range(NCORES):
        blocksA, blocksB = per_core[c]
        idxA, slotA = build_half(blocksA, TA)
        idxB, slotB = build_half(blocksB, TB)
        slotAB = np.concatenate([slotA, slotB]).reshape(TA_tot + TB_tot, P).T

        # in-degree bounds (dst-sorted cumulative positions), this core's shard
        deg_in = np.bincount(dst[core_of == c] - c * SH, minlength=SH)
        cum_in = np.concatenate([[0], np.cumsum(deg_in)]).astype(np.float32)

        # out-degree bounds for this shard, node-partition-wrapped [128, NBLK]
        lo = np.zeros(NBLK * P, np.float32)
        hi = np.zeros(NBLK * P, np.float32)
        lo[:SH] = cum_out[c * SH: (c + 1) * SH]
        hi[:SH] = cum_out[c * SH + 1: (c + 1) * SH + 1]
        b_out_lo = lo.reshape(NBLK, P).T.copy()
        b_out_hi = hi.reshape(NBLK, P).T.copy()

        # layer-1 features, block-contiguous: [NBLK, 128, KCH, 128]
        fb = np.zeros((NBLK, P, KCH, P), np.float32)
        shard = featT[:, c * SH:(c + 1) * SH]
        for nb in range(NBLK):
            m = P if nb < NBLK - 1 else LASTM
            fb[nb, :, :, :m] = (shard[:, nb * P: nb * P + m]
                                .reshape(KCH, P, m).transpose(1, 0, 2))

        in_maps.append({
            "featB": fb.astype(BF16),
            "W1c": W1p.reshape(KCH, P, HID).transpose(1, 0, 2).astype(BF16).copy(),
            "W2c": W2.astype(BF16), "W3c": W3.astype(BF16),
            "Wfcc": Wfc.astype(BF16),
            "b1c": b1.reshape(HID, 1).astype(np.float32),
            "b2c": b2.reshape(HID, 1).astype(np.float32),
            "b3c": b3.reshape(HID, 1).astype(np.float32),
            "bfcc": bfc.reshape(OUT_F, 1).astype(np.float32),
            "iota": np.tile(np.arange(P, dtype=np.float32).astype(BF16), (P, 1)),
            "ones_f": np.ones((1, P), np.float32),
            "idxA": _wrap_idx16(idxA), "idxB": _wrap_idx16(idxB),
            "slotAB": slotAB.astype(BF16),
            "b_in_lo": cum_in[:SH].reshape(1, SH),
            "b_in_hi": cum_in[1:SH + 1].reshape(1, SH),
            "b_out_lo": b_out_lo, "b_out_hi": b_out_hi,
        })

    sched = {"TA": TA.tolist(), "TB": TB.tolist(),
             "TA_tot": TA_tot, "TB_tot": TB_tot}
    return in_maps, sched


# ------------------------------------------------------------- device program

def _build(sched):
    TA, TB = sched["TA"], sched["TB"]
    TA_tot, TB_tot = sched["TA_tot"], sched["TB_tot"]
    EA, EB = TA_tot * P, TB_tot * P
    ncA = (TA_tot + CHUNK_TILES - 1) // CHUNK_TILES
    ncB = (TB_tot + CHUNK_TILES - 1) // CHUNK_TILES

    nc = bacc.Bacc("TRN2", target_bir_lowering=False, debug=False,
                   num_devices=NCORES, num_swdge_queues=4)

    def din(name, shape, dt):
        return nc.dram_tensor(name, shape, dt, kind="ExternalInput")

    h = {
        "featB": din("featB", [NBLK, P, KCH, P], BF),
        "W1c": din("W1c", [P, KCH, HID], BF),
        "W2c": din("W2c", [HID, HID], BF),
        "W3c": din("W3c", [HID, HID], BF),
        "Wfcc": din("Wfcc", [HID, OUT_F], BF),
        "b1c": din("b1c", [HID, 1], F32),
        "b2c": din("b2c", [HID, 1], F32),
        "b3c": din("b3c", [HID, 1], F32),
        "bfcc": din("bfcc", [OUT_F, 1], F32),
        "iota": din("iota", [P, P], BF),
        "ones_f": din("ones_f", [1, P], F32),
        "idxA": din("idxA", [P, EA // 16], I16),
        "idxB": din("idxB", [P, EB // 16], I16),
        "slotAB": din("slotAB", [P, TA_tot + TB_tot], BF),
        "b_in_lo": din("b_in_lo", [1, SH], F32),
        "b_in_hi": din("b_in_hi", [1, SH], F32),
        "b_out_lo": din("b_out_lo", [P, NBLK], F32),
        "b_out_hi": din("b_out_hi", [P, NBLK], F32),
    }
    out_fc = nc.dram_tensor("out_fc", [OUT_F, SH], F32, kind="ExternalOutput")
    # double-buffered across layers to break WAR hazards in the pipeline
    Tshard = [nc.dram_tensor(f"Tshard{i}", [SH, HID], BF) for i in range(2)]
    Tfull = [nc.dram_tensor(f"Tfull{i}", [N_NODES, HID], BF,
                            addr_space="Shared") for i in range(2)]

    with tile.TileContext(nc) as tc, \
         tc.tile_pool(name="const", bufs=1) as cp, \
         tc.tile_pool(name="state", bufs=1) as statep, \
         tc.tile_pool(name="feat", bufs=10) as featp, \
         tc.tile_pool(name="msgA", bufs=10) as msgAp, \
         tc.tile_pool(name="msgB", bufs=10) as msgBp, \
         tc.tile_pool(name="sTA", bufs=6) as sTAp, \
         tc.tile_pool(name="sTB", bufs=6) as sTBp, \
         tc.tile_pool(name="stage", bufs=4) as stagep, \
         tc.tile_pool(name="ps_agg", bufs=4, space="PSUM") as ps_agg, \
         tc.tile_pool(name="ps_dense", bufs=2, space="PSUM") as ps_dense, \
         tc.tile_pool(name="ps_misc", bufs=1, space="PSUM") as ps_misc, \
         tc.tile_pool(name="ps_fc", bufs=1, space="PSUM") as ps_fc:

        def load(name, shape, dt):
            t = cp.tile(shape, dt, tag=name)
            nc.sync.dma_start(out=t[:], in_=h[name][:])
            return t

        W1s = load("W1c", [P, KCH, HID], BF)
        W2s = load("W2c", [HID, HID], BF)
        W3s = load("W3c", [HID, HID], BF)
        Wfcs = load("Wfcc", [HID, OUT_F], BF)
        b1s = load("b1c", [HID, 1], F32)
        b2s = load("b2c", [HID, 1], F32)
        b3s = load("b3c", [HID, 1], F32)
        bfcs = load("bfcc", [OUT_F, 1], F32)
        iota = load("iota", [P, P], BF)
        ones_f = load("ones_f", [1, P], F32)
        idxA = load("idxA", [P, EA // 16], I16)
        idxB = load("idxB", [P, EB // 16], I16)
        slotAB = load("slotAB", [P, TA_tot + TB_tot], BF)
        bol = load("b_out_lo", [P, NBLK], F32)
        boh = load("b_out_hi", [P, NBLK], F32)

        # degrees -> r = 1/sqrt(max(deg,1))
        r_out = cp.tile([P, NBLK], F32, tag="r_out")
        nc.vector.tensor_sub(out=r_out[:], in0=boh[:], in1=bol[:])
        nc.vector.tensor_scalar_max(out=r_out[:], in0=r_out[:], scalar1=1.0)
        nc.scalar.activation(r_out[:], r_out[:], mybir.ActivationFunctionType.Sqrt)
        nc.vector.reciprocal(out=r_out[:], in_=r_out[:])

        # r_in = 1/sqrt(max(deg_in,1)), computed in 512-col chunks and
        # broadcast to [128, SH] via K=1 matmuls
        r_in_b = statep.tile([P, SH], F32, tag="r_in_b")
        for j in range(0, SH, 512):
            w = min(512, SH - j)
            blo = stagep.tile([1, 512], F32, tag="blo")
            bhi = stagep.tile([1, 512], F32, tag="bhi")
            nc.sync.dma_start(out=blo[:, :w], in_=h["b_in_lo"][:, j:j + w])
            nc.sync.dma_start(out=bhi[:, :w], in_=h["b_in_hi"][:, j:j + w])
            nc.vector.tensor_sub(out=blo[:, :w], in0=bhi[:, :w], in1=blo[:, :w])
            nc.vector.tensor_scalar_max(out=blo[:, :w], in0=blo[:, :w], scalar1=1.0)
            nc.scalar.activation(blo[:, :w], blo[:, :w],
                                 mybir.ActivationFunctionType.Sqrt)
            nc.vector.reciprocal_approx_fast(out=blo[:, :w], in_=blo[:, :w])
            pm = ps_misc.tile([P, 512], F32, space="PSUM")
            nc.tensor.matmul(pm[:, :w], lhsT=ones_f[:], rhs=blo[:, :w],
                             start=True, stop=True)
            nc.vector.tensor_copy(out=r_in_b[:, j:j + w], in_=pm[:, :w])

        x1 = statep.tile([P, SH], BF, tag="x1")
        x2 = statep.tile([P, SH], BF, tag="x2")
        x3 = x1  # x1 dead once dense-2 has consumed it

        Copy = mybir.ActivationFunctionType.Copy
        Relu = mybir.ActivationFunctionType.Relu

        # ---- emitters ----------------------------------------------------

        def ag(l, half):
            """AllGather of Tshard[l%2] half -> Tfull[l%2] region.  Issued on
            the gpsimd queue (the only engine with collective_compute); its
            SEQ hold is brief (input rows are ready by the time the FIFO
            reaches it) and the collective itself runs on the CC cores."""
            tsh, tf = Tshard[l % 2], Tfull[l % 2]
            nc.gpsimd.collective_compute(
                "AllGather", mybir.AluOpType.bypass,
                replica_groups=[list(range(NCORES))],
                ins=[tsh[half * HSH:(half + 1) * HSH, :].opt()],
                outs=[tf[half * HALF:(half + 1) * HALF, :].opt()])

        def emit_gather(l, tot_tiles, idx_t, half, pool, k, q):
            nt = min(CHUNK_TILES, tot_tiles - k * CHUNK_TILES)
            mt = pool.tile([P, nt, HID], BF, tag="msg")
            nidx = nt * P
            c0 = k * CHUNK_TILES * P // 16
            base = Tfull[l % 2][half * HALF:(half + 1) * HALF, :]
            nc.gpsimd.dma_gather(
                out_ap=mt[:], in_ap=base, idxs_ap=idx_t[:, c0:c0 + nidx // 16],
                num_idxs=nidx, num_idxs_reg=nidx, elem_size=HID,
                queue_num=q)
            return mt

        def gathers_layer(l, a_head=4):
            """Emit layer l's gather chunks.  A-half chunks ride SWDGE queues
            0/1, B-half 2/3, interleaved to match per-block A+B consumption
            (finite msg pools would deadlock on an unmixed order).  The first
            a_head A chunks go out unmixed so queues 0/1 fill while the B
            half's AllGather (whose sem blocks the Pool SEQ at the first B
            chunk) is still in flight."""
            mA, mB = [], []
            for k in range(a_head):
                if k < ncA:
                    mA.append(emit_gather(l, TA_tot, idxA, 0, msgAp, k, k % 2))
            for k in range(max(ncA, ncB)):
                if a_head + k < ncA:
                    ka = a_head + k
                    mA.append(emit_gather(l, TA_tot, idxA, 0, msgAp, ka, ka % 2))
                if k < ncB:
                    mB.append(emit_gather(l, TB_tot, idxB, 1, msgBp, k, 2 + k % 2))
            return mA, mB

        def st_group(pool, slot_off, g, tot_tiles):
            nt = min(ST_GROUP, tot_tiles - g * ST_GROUP)
            t = pool.tile([P, ST_GROUP, P], BF, tag="sT")
            sl = slotAB[:, slot_off + g * ST_GROUP: slot_off + g * ST_GROUP + nt]
            nc.vector.tensor_tensor(
                out=t[:, :nt, :],
                in0=sl.unsqueeze(2).to_broadcast([P, nt, P]),
                in1=iota[:].unsqueeze(1).to_broadcast([P, nt, P]),
                op=mybir.AluOpType.is_equal)
            return t

        def dense_block(xin, Wt, nb, tsh):
            """table row block: Tshard[nb] = r_out * (x[:,blk]^T @ W), bf16"""
            m = P if nb < NBLK - 1 else LASTM
            ps = ps_dense.tile([P, HID], F32, space="PSUM")
            nc.tensor.matmul(ps[:m, :], lhsT=xin[:, nb * P:nb * P + m],
                             rhs=Wt[:], start=True, stop=True)
            st = stagep.tile([P, HID], BF, tag="st")
            nc.scalar.activation(st[:m, :], ps[:m, :], Copy,
                                 scale=r_out[:m, nb:nb + 1])
            nc.scalar.dma_start(out=tsh[nb * P:nb * P + m, :], in_=st[:m, :])

        def aggregate_layer(l, mA, mB, bias, xout, post_block):
            """segment-sum layer l's gathered messages; per dst block apply
            r_in + bias + relu into xout.  post_block(nb) (lagged LAG blocks)
            emits the follow-on per-block work (next dense / residual add)."""
            sA, sB = {}, {}
            tAc = tBc = 0
            for b in range(NBLK):
                ps = ps_agg.tile([P, P], F32, space="PSUM")
                tot = TA[b] + TB[b]
                i = 0
                for (cnt, cur, msgs, sTs, pool, soff, ttot) in (
                        (TA[b], tAc, mA, sA, sTAp, 0, TA_tot),
                        (TB[b], tBc, mB, sB, sTBp, TA_tot, TB_tot)):
                    for t in range(cur, cur + cnt):
                        g = t // ST_GROUP
                        if g not in sTs:
                            sTs[g] = st_group(pool, soff, g, ttot)
                        nc.tensor.matmul(
                            ps[:],
                            lhsT=msgs[t // CHUNK_TILES][:, t % CHUNK_TILES, :],
                            rhs=sTs[g][:, t % ST_GROUP, :],
                            start=(i == 0), stop=(i == tot - 1))
                        i += 1
                tAc += TA[b]
                tBc += TB[b]
                m = P if b < NBLK - 1 else LASTM
                stg = stagep.tile([P, P], F32, tag="stg")
                nc.vector.tensor_mul(out=stg[:, :m], in0=ps[:, :m],
                                     in1=r_in_b[:, b * P:b * P + m])
                nc.scalar.activation(xout[:, b * P:b * P + m], stg[:, :m],
                                     Relu, bias=bias[:])
                if post_block is not None and b >= LAG:
                    post_block(b - LAG)
            if post_block is not None:
                for nb in range(NBLK - LAG, NBLK):
                    post_block(nb)

        # ---- layer 1 dense front (features streamed from DRAM) -----------
        for nb in range(NBLK):
            m = P if nb < NBLK - 1 else LASTM
            ft = featp.tile([P, KCH, P], BF, tag="ft")
            nc.sync.dma_start(out=ft[:], in_=h["featB"][nb])
            ps = ps_dense.tile([P, HID], F32, space="PSUM")
            for k in range(KCH):
                nc.tensor.matmul(ps[:m, :],
                                 lhsT=ft[:, k, :m],
                                 rhs=W1s[:, k, :],
                                 start=(k == 0), stop=(k == KCH - 1))
            st = stagep.tile([P, HID], BF, tag="st")
            nc.scalar.activation(st[:m, :], ps[:m, :], Copy,
                                 scale=r_out[:m, nb:nb + 1])
            nc.scalar.dma_start(out=Tshard[1][nb * P:nb * P + m, :], in_=st[:m, :])

        # Per layer: [AG half-0][AG half-1][gather chunks].  The AG pair is
        # FIFO-first on the gpsimd queue, so AG(l) of the first half fires as
        # soon as dense rows [0,HSH) land (mid dense / mid previous gather
        # stream) and the second half's AG hides behind the A-head chunks.
        ag(1, 0)
        ag(1, 1)
        mA1, mB1 = gathers_layer(1)
        aggregate_layer(
            1, mA1, mB1, b1s, x1,
            post_block=lambda nb: dense_block(x1, W2s, nb, Tshard[0]))

        ag(2, 0)
        ag(2, 1)
        mA2, mB2 = gathers_layer(2)
        aggregate_layer(
            2, mA2, mB2, b2s, x2,
            post_block=lambda nb: dense_block(x2, W3s, nb, Tshard[1]))

        ag(3, 0)
        ag(3, 1)
        mA3, mB3 = gathers_layer(3)

        # ---- layer 3 agg -> x3; fused residual x3 += x2 (both >= 0)
        def resid(nb):
            m = P if nb < NBLK - 1 else LASTM
            nc.vector.tensor_add(out=x3[:, nb * P:nb * P + m],
                                 in0=x3[:, nb * P:nb * P + m],
                                 in1=x2[:, nb * P:nb * P + m])
        aggregate_layer(3, mA3, mB3, b3s, x3, post_block=resid)

        # ---- FC head, feature-major: out[41, SH] = Wfc^T @ x4 + bfc
        for j in range(0, SH, 512):
            w = min(512, SH - j)
            ps = ps_fc.tile([OUT_F, 512], F32, space="PSUM")
            nc.tensor.matmul(ps[:, :w], lhsT=Wfcs[:], rhs=x3[:, j:j + w],
                             start=True, stop=True)
            st = stagep.tile([OUT_F, 512], F32, tag="stf")
            nc.vector.tensor_scalar_add(out=st[:, :w], in0=ps[:, :w],
                                        scalar1=bfcs[:])
            nc.sync.dma_start(out=out_fc[:, j:j + w], in_=st[:, :w])

    nc.compile()
    return nc


_CACHED = None


def kernel(**inputs):
    global _CACHED
    in_maps, sched = _preprocess(**inputs)
    if _CACHED is None or _CACHED[1] != sched:
        _CACHED = (_build(sched), sched)
    nc = _CACHED[0]
    res = run_bass_kernel_spmd(nc, in_maps, list(range(NCORES)))
    return np.concatenate(
        [np.asarray(res.results[c]["out_fc"], np.float32).T
         for c in range(NCORES)], 0)
